# revision 7
# baseline (speedup 1.0000x reference)
"""DRNN-Char (4-layer dilated QRNN + decoder) Trainium2 kernel, v2.

Sharding: data-parallel over batch. 16 batch rows across 8 cores = 2 rows/core.
Weights replicated. Each core computes its 2 rows fully on-chip.

v2 vs v1:
  - tanh-space pipeline: z~ = tanh(y_z), f = sig(y_f), o = sig(y_o);
    g = (f-1)*z~ (STT in place); scan: c = f*c - g (init 0) == fo-pool;
    H = o * c as a plain tensor_tensor MULT (f16 x f16 -> bf16, 2x DVE mode).
    No sign/scale folding needed anywhere; weights stay plain bf16.
  - gate activations read 4 PSUM banks in one ACTIVATE (N=2048): 3 ACTs
    per (layer,row,h-chunk) instead of 12 (scalar engine was 91% busy).
  - layer-0 bias folded into the fused embedding table w0f on device.
  - one-hot input DMA'd in 512-column chunks so the first matmul starts
    ~6us earlier; w0f built with 2 big PSUM tiles + 2 wide copies.
  - decoder drains 4 time-chunks per PSUM tile with one strided ACTIVATE.

Layout: activations feature-major [128, kch, T] bf16; gates f32 (cheap
strided scans), o-gate and scan output f16 (enables the 2x TT H-combine).
"""

import numpy as np
import ml_dtypes

EMB = 256
HID = 512
LAYERS = 4
VOCAB = 256
B = 16
T = 2048
NCORES = 8
BC = B // NCORES          # batch rows per core
HCH = HID // 128          # hidden chunks
MCH = 3 * HCH             # m-chunks of the 3H gate output

_cache = {}


def _build():
    """Build + compile the SPMD bass program (cached across calls)."""
    if "nc" in _cache:
        return _cache["nc"]

    import concourse.bass as bass
    import concourse.mybir as mybir
    import concourse.tile as tile
    from concourse import bacc

    f32 = mybir.dt.float32
    f16 = mybir.dt.float16
    bf16 = mybir.dt.bfloat16
    SIG = mybir.ActivationFunctionType.Sigmoid
    TANH = mybir.ActivationFunctionType.Tanh
    COPY = mybir.ActivationFunctionType.Copy
    MULT = mybir.AluOpType.mult
    SUB = mybir.AluOpType.subtract

    nc = bacc.Bacc(
        "TRN2",
        target_bir_lowering=False,
        debug=False,
        enable_asserts=False,
        num_devices=NCORES,
    )

    # ---- DRAM parameters (per-core inputs prepared by the host) ----
    oh_d = nc.dram_tensor("oh", [BC, 2, 128, T], bf16, kind="ExternalInput").ap()
    embt_d = nc.dram_tensor("embt", [2, 128, VOCAB], bf16, kind="ExternalInput").ap()
    w0_d = nc.dram_tensor("w0", [2, 128, 3 * HID], bf16, kind="ExternalInput").ap()
    b0_d = nc.dram_tensor("b0", [1, 3 * HID], bf16, kind="ExternalInput").ap()
    w_d = [w0_d] + [
        nc.dram_tensor(f"w{i}", [4, 128, 3 * HID], bf16, kind="ExternalInput").ap()
        for i in range(1, LAYERS)
    ]
    wd_d = nc.dram_tensor("wd", [4, 128, VOCAB], bf16, kind="ExternalInput").ap()
    bias_d = nc.dram_tensor("bias", [LAYERS, 128, MCH], f32, kind="ExternalInput").ap()
    decb_d = nc.dram_tensor("decb", [1, VOCAB], bf16, kind="ExternalInput").ap()
    out_d = nc.dram_tensor("out", [BC, T, VOCAB], f32, kind="ExternalOutput").ap()

    with tile.TileContext(nc) as tc:
        with (
            tc.tile_pool(name="consts", bufs=1) as consts,
            tc.tile_pool(name="acts", bufs=1) as acts,
            tc.tile_pool(name="gates", bufs=2) as gates,
            tc.tile_pool(name="outs", bufs=2) as outs,
            tc.tile_pool(name="psum", bufs=2, space="PSUM") as psum,
        ):
            # ---- resident tiles ----
            embt = consts.tile([128, 2, VOCAB], bf16, tag="embt", name="embt")
            w_sb = [consts.tile([128, 2, 3 * HID], bf16, tag="w0", name="w0")] + [
                consts.tile([128, 4, 3 * HID], bf16, tag=f"w{i}", name=f"w{i}")
                for i in range(1, LAYERS)
            ]
            w0f = consts.tile([128, 2, 3 * HID], bf16, tag="w0f", name="w0f")
            b0row = consts.tile([1, 3 * HID], bf16, tag="b0row", name="b0row")
            wd = consts.tile([128, 4, VOCAB], bf16, tag="wd", name="wd")
            bias = consts.tile([128, LAYERS, MCH], f32, tag="bias", name="bias")
            decb = consts.tile([1, VOCAB], bf16, tag="decb", name="decb")
            ones = consts.tile([1, 128], bf16, tag="ones", name="ones")

            # ping-pong activation buffers, [128, kchunk, T] bf16, per row
            xbuf = [acts.tile([128, 4, T], bf16, tag=f"x{r}", name=f"x{r}") for r in range(BC)]
            hbuf = [acts.tile([128, 4, T], bf16, tag=f"h{r}", name=f"h{r}") for r in range(BC)]

            # ---- input DMA (ordered by first use; one-hot chunked) ----
            nc.sync.dma_start(embt[:, 0, :], embt_d[0])
            nc.sync.dma_start(w_sb[0][:, 0, :], w0_d[0])
            nc.sync.dma_start(embt[:, 1, :], embt_d[1])
            nc.sync.dma_start(w_sb[0][:, 1, :], w0_d[1])
            nc.gpsimd.dma_start(b0row[:], b0_d[:])
            nc.gpsimd.memset(ones[:], 1.0)
            for q in range(4):  # first row's one-hot, in matmul consumption order
                for e in range(2):
                    nc.sync.dma_start(
                        xbuf[0][:, e, q * 512 : (q + 1) * 512],
                        oh_d[0, e, :, q * 512 : (q + 1) * 512],
                    )
            for q in range(4):
                for e in range(2):
                    nc.sync.dma_start(
                        xbuf[1][:, e, q * 512 : (q + 1) * 512],
                        oh_d[1, e, :, q * 512 : (q + 1) * 512],
                    )
            for li in range(LAYERS):
                nc.gpsimd.dma_start(bias[:, li, :], bias_d[li])
            for k in range(4):
                nc.sync.dma_start(w_sb[1][:, k, :], w_d[1][k])
            nc.gpsimd.dma_start(decb[:], decb_d[:])
            for i in range(2, LAYERS):
                for k in range(4):
                    nc.gpsimd.dma_start(w_sb[i][:, k, :], w_d[i][k])
            for k in range(4):
                nc.gpsimd.dma_start(wd[:, k, :], wd_d[k])

            # ---- layer-0 gate-value tables from the fused preact table ----
            # w0f preacts y[v,:] = (embT.T @ W0)[v,:] + b0; since layer 0's
            # input is one-hot, fold the activations in: w0t columns are
            # (g | f | sig(o)) with g = (sig(y_f)-1)*tanh(y_z), so layer 0's
            # matmuls directly produce scan inputs — no per-timestep STT.
            for m in range(2):  # vocab chunk (psum partition)
                psf = psum.tile([128, T], f32, tag="ps", name="psf")
                for ns in range(3):  # gate-column group: z, f, o
                    for k in range(2):
                        nc.tensor.matmul(
                            psf[:, ns * 512 : (ns + 1) * 512],
                            lhsT=embt[:, k, m * 128 : (m + 1) * 128],
                            rhs=w_sb[0][:, k, ns * 512 : (ns + 1) * 512],
                            start=(k == 0),
                            stop=False,
                        )
                    nc.tensor.matmul(
                        psf[:, ns * 512 : (ns + 1) * 512],
                        lhsT=ones[:],
                        rhs=b0row[:, ns * 512 : (ns + 1) * 512],
                        start=False,
                        stop=True,
                    )
                ztmp = gates.tile([128, 512], f16, tag="ztmp", name="ztmp")
                nc.scalar.activation(w0f[:, m, 512:1024], psf[:, 512:1024], SIG)
                nc.scalar.activation(w0f[:, m, 1024:1536], psf[:, 1024:1536], SIG)
                nc.scalar.activation(ztmp[:], psf[:, 0:512], TANH)
                nc.vector.scalar_tensor_tensor(
                    w0f[:, m, 0:512], w0f[:, m, 512:1024], 1.0, ztmp[:], SUB, MULT
                )

            # ---- QRNN layers ----
            for li in range(LAYERS):
                rate = 2 ** li
                kch = 2 if li == 0 else 4
                Wt = w0f if li == 0 else w_sb[li]
                # f16 scan I/O is fast when contiguous/stride-2, slow at
                # stride>=4; gate/carry dtypes chosen per layer accordingly.
                cdt = f16 if li <= 1 else f32
                gdt = f16 if li == 0 else f32
                for r in range(BC):
                    xin, hout = xbuf[r], hbuf[r]
                    for h in range(HCH):
                        gt = {}
                        for gi, gname in [(1, "f"), (2, "o"), (0, "z")]:
                            m = gi * HCH + h
                            ps = psum.tile([128, T], f32, tag="ps", name="ps")
                            for q in range(4):
                                for k in range(kch):
                                    nc.tensor.matmul(
                                        ps[:, q * 512 : (q + 1) * 512],
                                        lhsT=Wt[:, k, m * 128 : (m + 1) * 128],
                                        rhs=xin[:, k, q * 512 : (q + 1) * 512],
                                        start=(k == 0),
                                        stop=(k == kch - 1),
                                    )
                            dt = f16 if gname == "o" else gdt
                            g = gates.tile([128, T], dt, tag=gname, name=gname)
                            if li == 0:
                                # table matmul already produced gate VALUES
                                nc.scalar.activation(g[:], ps[:], COPY)
                            else:
                                nc.scalar.activation(
                                    g[:],
                                    ps[:],
                                    TANH if gname == "z" else SIG,
                                    bias=bias[:, li, m : m + 1],
                                )
                            gt[gname] = g
                        if li > 0:
                            # g = (f - 1) * z~   (in place in the z tile)
                            nc.vector.scalar_tensor_tensor(
                                gt["z"][:], gt["f"][:], 1.0, gt["z"][:], SUB, MULT
                            )
                        # c = f*c - g along time per dilated subsequence
                        cc = gates.tile([128, T], cdt, tag="cc", name="cc")
                        for j in range(rate):
                            sl = slice(j, T, rate)
                            nc.vector.tensor_tensor_scan(
                                cc[:, sl],
                                gt["f"][:, sl],
                                gt["z"][:, sl],
                                initial=0.0,
                                op0=MULT,
                                op1=SUB,
                            )
                        # H = sig(o) * c  (2x DVE mode when both are f16)
                        nc.vector.tensor_tensor(
                            hout[:, h, :], gt["o"][:], cc[:], MULT
                        )
                    xbuf[r], hbuf[r] = hbuf[r], xbuf[r]

            # ---- decoder: out[t, v] = H^T[:,t] . decW[:, v] + decb ----
            for r in range(BC):
                xin = xbuf[r]
                for mt4 in range(T // 512):  # 4 time-chunks per psum tile
                    ps = psum.tile([128, T], f32, tag="ps", name="ps")
                    for sub in range(4):
                        mt = mt4 * 4 + sub
                        for k in range(4):
                            nc.tensor.matmul(
                                ps[:, sub * 512 : sub * 512 + VOCAB],
                                lhsT=xin[:, k, mt * 128 : (mt + 1) * 128],
                                rhs=wd[:, k, :],
                                start=(k == 0),
                                stop=False,
                            )
                        nc.tensor.matmul(
                            ps[:, sub * 512 : sub * 512 + VOCAB],
                            lhsT=ones[:],
                            rhs=decb[:],
                            start=False,
                            stop=True,
                        )
                    ot = outs.tile([128, 4, VOCAB], f32, tag="ot", name="ot")
                    nc.scalar.activation(
                        ot[:],
                        ps[:].rearrange("p (s v) -> p s v", s=4)[:, :, 0:VOCAB],
                        COPY,
                    )
                    for sub in range(4):
                        mt = mt4 * 4 + sub
                        nc.sync.dma_start(
                            out_d[r, mt * 128 : (mt + 1) * 128, :], ot[:, sub, :]
                        )

    nc.compile()
    _cache["nc"] = nc
    return nc


def _prep_inputs(inputs):
    """Host-side sharding + layout/dtype prep. Returns in_maps for 8 cores."""
    bf = ml_dtypes.bfloat16
    x = np.asarray(inputs["x"]).astype(np.int64)
    emb = np.asarray(inputs["emb"], dtype=np.float32)
    Ws = [np.asarray(inputs[f"W{i}"], dtype=np.float32) for i in range(LAYERS)]
    bs = [np.asarray(inputs[f"b{i}"], dtype=np.float32) for i in range(LAYERS)]
    decW = np.asarray(inputs["decW"], dtype=np.float32)
    decb = np.asarray(inputs["decb"], dtype=np.float32)

    embt = np.ascontiguousarray(emb.T).reshape(2, 128, VOCAB).astype(bf)
    w0 = Ws[0].reshape(2, 128, 3 * HID).astype(bf)
    b0row = bs[0].reshape(1, 3 * HID).astype(bf)
    wrest = [Ws[i].reshape(4, 128, 3 * HID).astype(bf) for i in range(1, LAYERS)]
    wd = decW.reshape(4, 128, VOCAB).astype(bf)

    bias = np.zeros((LAYERS, 128, MCH), np.float32)
    for li in range(1, LAYERS):
        bias[li] = bs[li].reshape(MCH, 128).T  # [128, m]

    decbb = decb.reshape(1, VOCAB).astype(bf)

    in_maps = []
    for c in range(NCORES):
        oh = np.zeros((BC, VOCAB, T), bf)
        for r in range(BC):
            oh[r, x[BC * c + r], np.arange(T)] = 1.0
        in_maps.append(
            {
                "oh": oh.reshape(BC, 2, 128, T),
                "embt": embt,
                "w0": w0,
                "b0": b0row,
                "w1": wrest[0],
                "w2": wrest[1],
                "w3": wrest[2],
                "wd": wd,
                "bias": bias,
                "decb": decbb,
            }
        )
    return in_maps


def kernel(**inputs) -> np.ndarray:
    from concourse.bass_utils import run_bass_kernel_spmd

    try:  # reuse compiled NEFFs across kernel() invocations in one environment
        import jax, tempfile, os

        jax.config.update(
            "jax_compilation_cache_dir",
            os.environ.get("JAX_COMPILATION_CACHE_DIR")
            or os.path.join(tempfile.gettempdir(), "bass_jax_cache"),
        )
    except Exception:
        pass

    nc = _build()
    in_maps = _prep_inputs(inputs)
    res = run_bass_kernel_spmd(nc, in_maps, list(range(NCORES)))
    out = np.empty((B, T, VOCAB), np.float32)
    for c in range(NCORES):
        out[BC * c : BC * (c + 1)] = res.results[c]["out"]
    return out
